# revision 2
# baseline (speedup 1.0000x reference)
"""Trainium2 Bass kernel for nn_Bihomogeneous_k3.

Math (per batch row, complex z of dim 5 given as z_re/z_im):
  zz[m]   = z_i z_j z_k for the 35 triples i<=j<=k (lexicographic)
  prod    = zz[p] * conj(zz[q]) for the 630 pairs p<=q (lexicographic)
  out     = [Re(prod) (630) | Im(prod) on strict pairs p<q (595)]   -> [B, 1225]

Distribution: pure data parallel over 8 NeuronCores (batch sharded).

Per-core design (B_local = 16384 rows):
  Layout: batch-major megatiles [128 partitions, G=32 groups, features],
  row b = mt*4096 + p*32 + g. All f32.
  - zz stage: complex mults via c-packed tensor_tensor ops with broadcast
    (step-0) and reversed (negative-step) access patterns; on DVE + GPSIMD.
  - pair products: per p-block, one TT mult makes (Rp*R[p:], Ip*I[p:]),
    one makes (Ip*R[p+1:], -Rp*I[p+1:]) (using a negated-im copy of zz).
  - the + of the two product halves runs on the TensorEngine as two
    accumulating identity-weight matmuls into PSUM (exact for fp32).
  - ScalarE (ACT) copies PSUM -> SBUF output chunks; HWDGE DMAs write out.
Engines all land at ~60-90% of the ~225us/core HBM write roofline.
"""
import os
import sys

sys.path.insert(0, "/opt/trn_rl_repo")

import numpy as np

N = 5
NC = 8
B_FULL = 131072
B_LOCAL = B_FULL // NC
P = 128
G = 32
ROWS_PER_MT = P * G  # 4096
N_MT = B_LOCAL // ROWS_PER_MT  # 4

# ---- index tables (python-time constants) ----
TRIPLES = [(i, j, k) for i in range(N) for j in range(N) for k in range(N) if i <= j <= k]
M = len(TRIPLES)  # 35
WPAIRS = [(i, j) for i in range(N) for j in range(N, ) if False]  # placeholder
WPAIRS = [(i, j) for i in range(N) for j in range(i, N)]  # 15, lex order
WOFF = {}
_o = 0
for (i, j) in WPAIRS:
    WOFF[(i, j)] = _o
    _o += 1
# zz offsets: triples are (i,j) pairs each followed by k=j..4 (lex order)
ZOFF = {}
_o = 0
for (i, j) in WPAIRS:
    ZOFF[(i, j)] = _o
    _o += N - j
assert _o == M

# re block p covers output cols [REOFF[p], REOFF[p]+35-p); im after 630
REOFF = np.concatenate([[0], np.cumsum([M - p for p in range(M)])]).astype(int)
IMOFF = np.concatenate([[0], np.cumsum([M - 1 - p for p in range(M - 1)])]).astype(int)
N_RE = int(REOFF[M])        # 630
N_IM = int(IMOFF[M - 1])    # 595
N_OUT = N_RE + N_IM         # 1225

TC_MAX = 512 // G  # psum chunk width in t-columns (16)

# output column chunks (block-aligned). Each entry: (colbase, cols, blocks)
# where blocks is a list of ("re"/"im", p, block_col_base_in_chunk)
def _make_chunks(max_cols=448):
    blocks = []
    for p in range(M):
        blocks.append(("re", p, int(REOFF[p]), M - p))
    for p in range(M - 1):
        blocks.append(("im", p, N_RE + int(IMOFF[p]), M - 1 - p))
    chunks = []
    cur = []
    base = 0
    cols = 0
    for kind, p, cb, w in blocks:
        if cols + w > max_cols and cur:
            chunks.append((base, cols, cur))
            base = cb
            cols = 0
            cur = []
        cur.append((kind, p, cb - base, w))
        cols += w
    if cur:
        chunks.append((base, cols, cur))
    return chunks

CHUNKS = _make_chunks()


def _ap(base_ap, offset_elems, dims, bassmod):
    """Build a raw AP from a tile's base AP: dims = [[step, count], ...] in
    elements, offset_elems added to the base offset."""
    return bassmod.AP(tensor=base_ap.tensor, offset=base_ap.offset + offset_elems,
                      ap=[list(base_ap.ap[0])] + [list(d) for d in dims])


def build_bass(n_mt=N_MT, g=G):
    import concourse.bacc as bacc
    import concourse.bass as bass
    import concourse.tile as tile
    from concourse import mybir
    from contextlib import ExitStack

    f32 = mybir.dt.float32
    b_local = P * g * n_mt

    nc = bacc.Bacc(None)
    z_re_d = nc.dram_tensor("z_re", [b_local, N], f32, kind="ExternalInput")
    z_im_d = nc.dram_tensor("z_im", [b_local, N], f32, kind="ExternalInput")
    ident_d = nc.dram_tensor("ident", [P, P], f32, kind="ExternalInput")
    out_d = nc.dram_tensor("out", [b_local, N_OUT], f32, kind="ExternalOutput")

    tc_max = 512 // g

    # greedy DVE/GPS load balancing (units: ~DVE cycles)
    eng_load = {"v": 0.0, "g": 0.0}

    def pick(fd):
        # DVE: 58 + fd cycles ; GPS: ~80 + 2.08*fd
        cv = eng_load["v"] + 58 + fd
        cg = eng_load["g"] + 80 + 2.08 * fd
        if cv <= cg:
            eng_load["v"] = cv
            return nc.vector
        eng_load["g"] = cg
        return nc.gpsimd

    with tile.TileContext(nc) as tc:
        with ExitStack() as ctx:
            const_pool = ctx.enter_context(tc.tile_pool(name="const", bufs=1))
            zpool = ctx.enter_context(tc.tile_pool(name="zp", bufs=2))
            wpool = ctx.enter_context(tc.tile_pool(name="wp", bufs=2))
            zzpool = ctx.enter_context(tc.tile_pool(name="zzp", bufs=2))
            t1pool = ctx.enter_context(tc.tile_pool(name="t1p", bufs=2))
            trpool = ctx.enter_context(tc.tile_pool(name="trp", bufs=2))
            outpool = ctx.enter_context(tc.tile_pool(name="outp", bufs=2))
            psum_pool = ctx.enter_context(tc.tile_pool(name="ps", bufs=6, space="PSUM"))

            ident = const_pool.tile([P, P], f32)
            nc.sync.dma_start(out=ident, in_=ident_d[:, :])

            mult = mybir.AluOpType.mult
            add = mybir.AluOpType.add
            sub = mybir.AluOpType.subtract

            for mt in range(n_mt):
                r0 = mt * P * g
                # ---- load z: z2 [P, 2, g, N] (c outer) ----
                z2 = zpool.tile([P, 2, g, N], f32)
                src_re = z_re_d[r0:r0 + P * g, :].rearrange("(p g) f -> p g f", g=g)
                src_im = z_im_d[r0:r0 + P * g, :].rearrange("(p g) f -> p g f", g=g)
                nc.sync.dma_start(out=z2[:, 0, :, :], in_=src_re)
                nc.sync.dma_start(out=z2[:, 1, :, :], in_=src_im)
                zb = z2[:, :, :, :]  # base AP; free dims [2*g*N] strides: c=g*N, g=N, f=1
                cZ, gZ = g * N, N

                # ---- w stage: w2 [P, 2, g, 15] ----
                w2 = wpool.tile([P, 2, g, len(WPAIRS)], f32)
                wb = w2[:, :, :, :]
                cW, gW = g * len(WPAIRS), len(WPAIRS)
                for i in range(N):
                    ti_ = N - i
                    off = WOFF[(i, i)]
                    # m1 = (zre_i, zim_i) bcast * (zre[i:], zim[i:]) -> [P, g, 2, ti]
                    t1 = t1pool.tile([P, g, 2, N], f32)
                    t1b = t1[:, :, :, :]
                    in0 = _ap(zb, i, [[gZ, g], [cZ, 2], [0, ti_]], bass)
                    in1 = _ap(zb, i, [[gZ, g], [cZ, 2], [1, ti_]], bass)
                    o1 = _ap(t1b, 0, [[2 * N, g], [N, 2], [1, ti_]], bass)
                    pick(2 * g * ti_).tensor_tensor(out=o1, in0=in0, in1=in1, op=mult)
                    # w_re[i block] = m1[c0] - m1[c1]
                    a0 = _ap(t1b, 0, [[2 * N, g], [1, ti_]], bass)
                    a1 = _ap(t1b, N, [[2 * N, g], [1, ti_]], bass)
                    ow = _ap(wb, off, [[gW, g], [1, ti_]], bass)
                    pick(g * ti_).tensor_tensor(out=ow, in0=a0, in1=a1, op=sub)
                    # m2 = (zim_i, zre_i) bcast * (zre[i:], zim[i:])
                    t2 = t1pool.tile([P, g, 2, N], f32, tag="t2")
                    t2b = t2[:, :, :, :]
                    in0r = _ap(zb, cZ + i, [[gZ, g], [-cZ, 2], [0, ti_]], bass)
                    o2 = _ap(t2b, 0, [[2 * N, g], [N, 2], [1, ti_]], bass)
                    pick(2 * g * ti_).tensor_tensor(out=o2, in0=in0r, in1=in1, op=mult)
                    a0 = _ap(t2b, 0, [[2 * N, g], [1, ti_]], bass)
                    a1 = _ap(t2b, N, [[2 * N, g], [1, ti_]], bass)
                    ow = _ap(wb, cW + off, [[gW, g], [1, ti_]], bass)
                    pick(g * ti_).tensor_tensor(out=ow, in0=a0, in1=a1, op=add)

                # ---- zz stage: zz3 [P, 3, g, 35] (re, im, negim) ----
                zz3 = zzpool.tile([P, 3, g, M], f32)
                zzb = zz3[:, :, :, :]
                cA, gA = g * M, M
                for (i, j) in WPAIRS:
                    tk = N - j
                    pr = WOFF[(i, j)]
                    zo = ZOFF[(i, j)]
                    # m3 = (wre, wim) bcast * (zre[j:], zim[j:])
                    t3 = t1pool.tile([P, g, 2, N], f32, tag="t3")
                    t3b = t3[:, :, :, :]
                    in0 = _ap(wb, pr, [[gW, g], [cW, 2], [0, tk]], bass)
                    in1 = _ap(zb, j, [[gZ, g], [cZ, 2], [1, tk]], bass)
                    o3 = _ap(t3b, 0, [[2 * N, g], [N, 2], [1, tk]], bass)
                    pick(2 * g * tk).tensor_tensor(out=o3, in0=in0, in1=in1, op=mult)
                    a0 = _ap(t3b, 0, [[2 * N, g], [1, tk]], bass)
                    a1 = _ap(t3b, N, [[2 * N, g], [1, tk]], bass)
                    oz = _ap(zzb, zo, [[gA, g], [1, tk]], bass)
                    pick(g * tk).tensor_tensor(out=oz, in0=a0, in1=a1, op=sub)
                    # m4 = (wim, wre) bcast * (zre[j:], zim[j:])
                    t4 = t1pool.tile([P, g, 2, N], f32, tag="t4")
                    t4b = t4[:, :, :, :]
                    in0r = _ap(wb, cW + pr, [[gW, g], [-cW, 2], [0, tk]], bass)
                    o4 = _ap(t4b, 0, [[2 * N, g], [N, 2], [1, tk]], bass)
                    pick(2 * g * tk).tensor_tensor(out=o4, in0=in0r, in1=in1, op=mult)
                    a0 = _ap(t4b, 0, [[2 * N, g], [1, tk]], bass)
                    a1 = _ap(t4b, N, [[2 * N, g], [1, tk]], bass)
                    oz = _ap(zzb, cA + zo, [[gA, g], [1, tk]], bass)
                    pick(g * tk).tensor_tensor(out=oz, in0=a0, in1=a1, op=add)
                # negim slot: zz3[:,2] = -zz3[:,1]
                src = _ap(zzb, cA, [[gA, g], [1, M]], bass)
                dst = _ap(zzb, 2 * cA, [[gA, g], [1, M]], bass)
                nc.gpsimd.tensor_scalar_mul(out=dst, in0=src, scalar1=-1.0)

                # ---- products + PE adds + ACT copies, chunk by chunk ----
                for (colbase, cols, blist) in CHUNKS:
                    outc = outpool.tile([P, g, 448], f32)
                    ocb = outc[:, :, :]
                    gO = 448
                    for (kind, p, cb, w) in blist:
                        # tmp = one TT mult producing both product halves
                        tr = trpool.tile([P, g, 2, M], f32, tag="tr")
                        trb = tr[:, :, :, :]
                        if kind == "re":
                            # (Rp, Ip) bcast * (R[p:], I[p:])
                            in0 = _ap(zzb, p, [[gA, g], [cA, 2], [0, w]], bass)
                            in1 = _ap(zzb, p, [[gA, g], [cA, 2], [1, w]], bass)
                        else:
                            # (Ip, Rp) bcast * (R[p+1:], negI[p+1:])
                            in0 = _ap(zzb, cA + p, [[gA, g], [-cA, 2], [0, w]], bass)
                            in1 = _ap(zzb, p + 1, [[gA, g], [2 * cA, 2], [1, w]], bass)
                        omul = _ap(trb, 0, [[2 * M, g], [M, 2], [1, w]], bass)
                        pick(2 * g * w).tensor_tensor(out=omul, in0=in0, in1=in1, op=mult)
                        # PE adds in <=512-elem psum chunks; ACT drains
                        t0 = 0
                        while t0 < w:
                            tcw = min(tc_max, w - t0)
                            ps = psum_pool.tile([P, g, tcw], f32, tag="ps")
                            rhs0 = _ap(trb, t0, [[2 * M, g], [1, tcw]], bass)
                            rhs1 = _ap(trb, M + t0, [[2 * M, g], [1, tcw]], bass)
                            psb = ps[:, :, :]
                            pso = _ap(psb, 0, [[tcw, g], [1, tcw]], bass)
                            nc.tensor.matmul(pso, ident, rhs0, start=True, stop=False)
                            nc.tensor.matmul(pso, ident, rhs1, start=False, stop=True)
                            oc = _ap(ocb, cb + t0, [[gO, g], [1, tcw]], bass)
                            nc.scalar.copy(out=oc, in_=pso)
                            t0 += tcw
                    # DMA chunk out
                    dst = out_d[r0:r0 + P * g, colbase:colbase + cols].rearrange(
                        "(p g) f -> p g f", g=g)
                    nc.sync.dma_start(out=dst, in_=_ap(ocb, 0, [[gO, g], [1, cols]], bass))

    nc.finalize()
    return nc


_CACHED = {}


def _get_nc():
    if "nc" not in _CACHED:
        _CACHED["nc"] = build_bass()
    return _CACHED["nc"]


def kernel(z_re, z_im):
    from concourse.bass_utils import run_bass_kernel_spmd

    z_re = np.ascontiguousarray(np.asarray(z_re, dtype=np.float32))
    z_im = np.ascontiguousarray(np.asarray(z_im, dtype=np.float32))
    assert z_re.shape == (B_FULL, N), z_re.shape

    nc = _get_nc()
    ident = np.eye(P, dtype=np.float32)
    in_maps = []
    for c in range(NC):
        sl = slice(c * B_LOCAL, (c + 1) * B_LOCAL)
        in_maps.append({
            "z_re": np.ascontiguousarray(z_re[sl]),
            "z_im": np.ascontiguousarray(z_im[sl]),
            "ident": ident,
        })
    res = run_bass_kernel_spmd(nc, in_maps, core_ids=list(range(NC)))
    return np.concatenate([res.results[c]["out"] for c in range(NC)], axis=0)


# revision 6
# speedup vs baseline: 1.3880x; 1.3880x over previous
"""Trainium2 Bass kernel for nn_Bihomogeneous_k3.

Math (per batch row, complex z of dim 5 given as z_re/z_im):
  zz[m]   = z_i z_j z_k for the 35 triples i<=j<=k (lexicographic)
  prod    = zz[p] * conj(zz[q]) for the 630 pairs p<=q (lexicographic)
  out     = [Re(prod) (630) | Im(prod) on strict pairs p<q (595)]   -> [B, 1225]

Distribution: pure data parallel over 8 NeuronCores (batch sharded).

Per-core design (B_local = 16384 rows):
  Layout: batch-major megatiles [128 partitions, G=32 groups, features],
  row b = mt*4096 + p*32 + g. All f32.
  - zz stage: complex mults via c-packed tensor_tensor ops with broadcast
    (step-0) and reversed (negative-step) access patterns; on DVE + GPSIMD.
  - pair products: per p-block, one TT mult makes (Rp*R[p:], Ip*I[p:]),
    one makes (Ip*R[p+1:], -Rp*I[p+1:]) (using a negated-im copy of zz).
  - the + of the two product halves runs on the TensorEngine as two
    accumulating identity-weight matmuls into PSUM (exact for fp32).
  - ScalarE (ACT) copies PSUM -> SBUF output chunks; HWDGE DMAs write out.
Engines all land at ~60-90% of the ~225us/core HBM write roofline.
"""
import os
import sys

sys.path.insert(0, "/opt/trn_rl_repo")

import numpy as np

N = 5
NC = 8
B_FULL = 131072
B_LOCAL = B_FULL // NC
P = 128
G = 32
ROWS_PER_MT = P * G  # 4096
N_MT = B_LOCAL // ROWS_PER_MT  # 4

# ---- index tables (python-time constants) ----
TRIPLES = [(i, j, k) for i in range(N) for j in range(N) for k in range(N) if i <= j <= k]
M = len(TRIPLES)  # 35
WPAIRS = [(i, j) for i in range(N) for j in range(N, ) if False]  # placeholder
WPAIRS = [(i, j) for i in range(N) for j in range(i, N)]  # 15, lex order
WOFF = {}
_o = 0
for (i, j) in WPAIRS:
    WOFF[(i, j)] = _o
    _o += 1
# zz offsets: triples are (i,j) pairs each followed by k=j..4 (lex order)
ZOFF = {}
_o = 0
for (i, j) in WPAIRS:
    ZOFF[(i, j)] = _o
    _o += N - j
assert _o == M

# re block p covers output cols [REOFF[p], REOFF[p]+35-p); im after 630
REOFF = np.concatenate([[0], np.cumsum([M - p for p in range(M)])]).astype(int)
IMOFF = np.concatenate([[0], np.cumsum([M - 1 - p for p in range(M - 1)])]).astype(int)
N_RE = int(REOFF[M])        # 630
N_IM = int(IMOFF[M - 1])    # 595
N_OUT = N_RE + N_IM         # 1225

TC_MAX = 512 // G  # psum chunk width in t-columns (16)

# output column chunks (block-aligned). Each entry: (colbase, cols, blocks)
# where blocks is a list of ("re"/"im", p, block_col_base_in_chunk)
def _make_chunks(max_cols=448):
    blocks = []
    for p in range(M):
        blocks.append(("re", p, int(REOFF[p]), M - p))
    for p in range(M - 1):
        blocks.append(("im", p, N_RE + int(IMOFF[p]), M - 1 - p))
    chunks = []
    cur = []
    base = 0
    cols = 0
    for kind, p, cb, w in blocks:
        if cols + w > max_cols and cur:
            chunks.append((base, cols, cur))
            base = cb
            cols = 0
            cur = []
        cur.append((kind, p, cb - base, w))
        cols += w
    if cur:
        chunks.append((base, cols, cur))
    return chunks

CHUNKS = _make_chunks()


def _ap(base_ap, offset_elems, dims, bassmod):
    """Build a raw AP from a tile's base AP: dims = [[step, count], ...] in
    elements, offset_elems added to the base offset."""
    return bassmod.AP(tensor=base_ap.tensor, offset=base_ap.offset + offset_elems,
                      ap=[list(base_ap.ap[0])] + [list(d) for d in dims])


def build_bass(n_mt=N_MT, g=G):
    import concourse.bacc as bacc
    import concourse.bass as bass
    import concourse.tile as tile
    from concourse import mybir
    from contextlib import ExitStack

    f32 = mybir.dt.float32
    b_local = P * g * n_mt

    nc = bacc.Bacc(None)
    z_re_d = nc.dram_tensor("z_re", [b_local, N], f32, kind="ExternalInput")
    z_im_d = nc.dram_tensor("z_im", [b_local, N], f32, kind="ExternalInput")
    ident_d = nc.dram_tensor("ident", [P, P], f32, kind="ExternalInput")
    out_d = nc.dram_tensor("out", [b_local, N_OUT], f32, kind="ExternalOutput")

    tc_max = 512 // g

    # greedy DVE/GPS load balancing (units: ~DVE cycles)
    eng_load = {"v": 0.0, "g": 0.0}

    def pick(fd):
        # DVE: 58 + fd cycles ; GPS: ~80 + 2.08*fd
        cv = eng_load["v"] + 58 + fd
        cg = eng_load["g"] + 80 + 2.08 * fd
        if cv <= cg:
            eng_load["v"] = cv
            return nc.vector
        eng_load["g"] = cg
        return nc.gpsimd

    with tile.TileContext(nc) as tc:
        with ExitStack() as ctx:
            const_pool = ctx.enter_context(tc.tile_pool(name="const", bufs=1))
            zpool = ctx.enter_context(tc.tile_pool(name="zp", bufs=2))
            wpool = ctx.enter_context(tc.tile_pool(name="wp", bufs=2))
            zzpool = ctx.enter_context(tc.tile_pool(name="zzp", bufs=2))
            t1pool = ctx.enter_context(tc.tile_pool(name="t1p", bufs=2))
            trpool = ctx.enter_context(tc.tile_pool(name="trp", bufs=2))
            outpool = ctx.enter_context(tc.tile_pool(name="outp", bufs=2))
            psum_pool = ctx.enter_context(tc.tile_pool(name="ps", bufs=2, space="PSUM"))

            ident = const_pool.tile([P, P], f32)
            nc.sync.dma_start(out=ident, in_=ident_d[:, :])

            mult = mybir.AluOpType.mult
            add = mybir.AluOpType.add
            sub = mybir.AluOpType.subtract

            for mt in range(n_mt):
                r0 = mt * P * g
                # ---- load z: z2 [P, 2, g, N] (c outer) ----
                z2 = zpool.tile([P, 2, g, N], f32)
                src_re = z_re_d[r0:r0 + P * g, :].rearrange("(p g) f -> p g f", g=g)
                src_im = z_im_d[r0:r0 + P * g, :].rearrange("(p g) f -> p g f", g=g)
                nc.sync.dma_start(out=z2[:, 0, :, :], in_=src_re)
                nc.sync.dma_start(out=z2[:, 1, :, :], in_=src_im)
                zb = z2[:, :, :, :]  # base AP; free dims [2*g*N] strides: c=g*N, g=N, f=1
                cZ, gZ = g * N, N

                # ---- w stage: w2 [P, 2, g, 15] ----
                w2 = wpool.tile([P, 2, g, len(WPAIRS)], f32)
                wb = w2[:, :, :, :]
                cW, gW = g * len(WPAIRS), len(WPAIRS)
                for i in range(N):
                    ti_ = N - i
                    off = WOFF[(i, i)]
                    # m1 = (zre_i, zim_i) bcast * (zre[i:], zim[i:]) -> [P, g, 2, ti]
                    t1 = t1pool.tile([P, g, 2, N], f32)
                    t1b = t1[:, :, :, :]
                    in0 = _ap(zb, i, [[gZ, g], [cZ, 2], [0, ti_]], bass)
                    in1 = _ap(zb, i, [[gZ, g], [cZ, 2], [1, ti_]], bass)
                    o1 = _ap(t1b, 0, [[2 * N, g], [N, 2], [1, ti_]], bass)
                    pick(2 * g * ti_).tensor_tensor(out=o1, in0=in0, in1=in1, op=mult)
                    # w_re[i block] = m1[c0] - m1[c1]
                    a0 = _ap(t1b, 0, [[2 * N, g], [1, ti_]], bass)
                    a1 = _ap(t1b, N, [[2 * N, g], [1, ti_]], bass)
                    ow = _ap(wb, off, [[gW, g], [1, ti_]], bass)
                    pick(g * ti_).tensor_tensor(out=ow, in0=a0, in1=a1, op=sub)
                    # m2 = (zim_i, zre_i) bcast * (zre[i:], zim[i:])
                    t2 = t1pool.tile([P, g, 2, N], f32, tag="t2")
                    t2b = t2[:, :, :, :]
                    in0r = _ap(zb, cZ + i, [[gZ, g], [-cZ, 2], [0, ti_]], bass)
                    o2 = _ap(t2b, 0, [[2 * N, g], [N, 2], [1, ti_]], bass)
                    pick(2 * g * ti_).tensor_tensor(out=o2, in0=in0r, in1=in1, op=mult)
                    a0 = _ap(t2b, 0, [[2 * N, g], [1, ti_]], bass)
                    a1 = _ap(t2b, N, [[2 * N, g], [1, ti_]], bass)
                    ow = _ap(wb, cW + off, [[gW, g], [1, ti_]], bass)
                    pick(g * ti_).tensor_tensor(out=ow, in0=a0, in1=a1, op=add)

                # ---- zz stage: zz3 [P, 3, g, 35] (re, im, negim) ----
                zz3 = zzpool.tile([P, 3, g, M], f32)
                zzb = zz3[:, :, :, :]
                cA, gA = g * M, M
                for (i, j) in WPAIRS:
                    tk = N - j
                    pr = WOFF[(i, j)]
                    zo = ZOFF[(i, j)]
                    # m3 = (wre, wim) bcast * (zre[j:], zim[j:])
                    t3 = t1pool.tile([P, g, 2, N], f32, tag="t3")
                    t3b = t3[:, :, :, :]
                    in0 = _ap(wb, pr, [[gW, g], [cW, 2], [0, tk]], bass)
                    in1 = _ap(zb, j, [[gZ, g], [cZ, 2], [1, tk]], bass)
                    o3 = _ap(t3b, 0, [[2 * N, g], [N, 2], [1, tk]], bass)
                    pick(2 * g * tk).tensor_tensor(out=o3, in0=in0, in1=in1, op=mult)
                    a0 = _ap(t3b, 0, [[2 * N, g], [1, tk]], bass)
                    a1 = _ap(t3b, N, [[2 * N, g], [1, tk]], bass)
                    oz = _ap(zzb, zo, [[gA, g], [1, tk]], bass)
                    pick(g * tk).tensor_tensor(out=oz, in0=a0, in1=a1, op=sub)
                    # m4 = (wim, wre) bcast * (zre[j:], zim[j:])
                    t4 = t1pool.tile([P, g, 2, N], f32, tag="t4")
                    t4b = t4[:, :, :, :]
                    in0r = _ap(wb, cW + pr, [[gW, g], [-cW, 2], [0, tk]], bass)
                    o4 = _ap(t4b, 0, [[2 * N, g], [N, 2], [1, tk]], bass)
                    pick(2 * g * tk).tensor_tensor(out=o4, in0=in0r, in1=in1, op=mult)
                    a0 = _ap(t4b, 0, [[2 * N, g], [1, tk]], bass)
                    a1 = _ap(t4b, N, [[2 * N, g], [1, tk]], bass)
                    oz = _ap(zzb, cA + zo, [[gA, g], [1, tk]], bass)
                    pick(g * tk).tensor_tensor(out=oz, in0=a0, in1=a1, op=add)
                # negim slot: zz3[:,2] = -zz3[:,1]
                src = _ap(zzb, cA, [[gA, g], [1, M]], bass)
                dst = _ap(zzb, 2 * cA, [[gA, g], [1, M]], bass)
                nc.gpsimd.tensor_scalar_mul(out=dst, in0=src, scalar1=-1.0)

                # ---- products: DVE half -> PSUM, PE accumulates SBUF half,
                # ---- ACT drains PSUM -> out chunk, chunk DMA'd out
                for (colbase, cols, blist) in CHUNKS:
                    outc = outpool.tile([P, g, 448], f32)
                    ocb = outc[:, :, :]
                    gO = 448
                    for (kind, p, cb, w) in blist:
                        pst = psum_pool.tile([P, g, w], f32, tag="ps")
                        psb = pst[:, :, :]
                        tr = trpool.tile([P, g, M], f32, tag="tr")
                        trb = tr[:, :, :]
                        if kind == "re":
                            # psum half: Rp * R[p:] ; sbuf half: Ip * I[p:]
                            inA0 = _ap(zzb, p, [[gA, g], [0, w]], bass)
                            inA1 = _ap(zzb, p, [[gA, g], [1, w]], bass)
                            inB0 = _ap(zzb, cA + p, [[gA, g], [0, w]], bass)
                            inB1 = _ap(zzb, cA + p, [[gA, g], [1, w]], bass)
                        else:
                            # psum half: Ip * R[p+1:] ; sbuf half: Rp * (-I[p+1:])
                            inA0 = _ap(zzb, cA + p, [[gA, g], [0, w]], bass)
                            inA1 = _ap(zzb, p + 1, [[gA, g], [1, w]], bass)
                            inB0 = _ap(zzb, p, [[gA, g], [0, w]], bass)
                            inB1 = _ap(zzb, 2 * cA + p + 1, [[gA, g], [1, w]], bass)
                        outA = _ap(psb, 0, [[w, g], [1, w]], bass)
                        eng_load["v"] += 120 + g * w
                        nc.vector.tensor_tensor(out=outA, in0=inA0, in1=inA1, op=mult)
                        outB = _ap(trb, 0, [[w, g], [1, w]], bass)  # packed flat [g*w]
                        pick(g * w).tensor_tensor(out=outB, in0=inB0, in1=inB1, op=mult)
                        # PE: accumulate sbuf half onto psum, 512-elem flat chunks
                        flat = g * w
                        t0 = 0
                        while t0 < flat:
                            tcw = min(512, flat - t0)
                            rhs = _ap(trb, t0, [[1, tcw]], bass)
                            pso = _ap(psb, t0, [[1, tcw]], bass)
                            nc.tensor.matmul(pso, ident, rhs, start=False, stop=True,
                                             skip_group_check=True)
                            t0 += tcw
                        # ACT: drain psum block -> out chunk columns
                        oc = _ap(ocb, cb, [[gO, g], [1, w]], bass)
                        nc.scalar.copy(out=oc, in_=_ap(psb, 0, [[w, g], [1, w]], bass))
                    # DMA chunk out
                    dst = out_d[r0:r0 + P * g, colbase:colbase + cols].rearrange(
                        "(p g) f -> p g f", g=g)
                    nc.sync.dma_start(out=dst, in_=_ap(ocb, 0, [[gO, g], [1, cols]], bass))

    nc.finalize()
    return nc


_CACHED = {}


def _get_nc():
    if "nc" not in _CACHED:
        _CACHED["nc"] = build_bass()
    return _CACHED["nc"]


def kernel(z_re, z_im):
    from concourse.bass_utils import run_bass_kernel_spmd

    z_re = np.ascontiguousarray(np.asarray(z_re, dtype=np.float32))
    z_im = np.ascontiguousarray(np.asarray(z_im, dtype=np.float32))
    assert z_re.shape == (B_FULL, N), z_re.shape

    nc = _get_nc()
    ident = np.eye(P, dtype=np.float32)
    in_maps = []
    for c in range(NC):
        sl = slice(c * B_LOCAL, (c + 1) * B_LOCAL)
        in_maps.append({
            "z_re": np.ascontiguousarray(z_re[sl]),
            "z_im": np.ascontiguousarray(z_im[sl]),
            "ident": ident,
        })
    res = run_bass_kernel_spmd(nc, in_maps, core_ids=list(range(NC)))
    return np.concatenate([res.results[c]["out"] for c in range(NC)], axis=0)
